# revision 4
# baseline (speedup 1.0000x reference)
"""XNOR-Net conv2d kernel for Trainium2.

Computes conv2d(sign(x), sign(W), stride=1, pad=1) * alpha for
x:(32,256,56,56) f32, W:(256,256,3,3) f32, alpha:(256,1,1) f32.

Strategy: data-parallel over batch (4 images per core x 8 cores).
Per core, implicit GEMM: sign(x) is materialized in SBUF as a
zero-padded bf16 image [128 part = C_in/2, 2 cg, 58 rows, 64 row-stride];
each 3x3 tap is a [K=128, M=128] x [K=128, N=448] matmul accumulated in
PSUM (18 matmuls per output tile: 2 C_in groups x 9 taps). Sign values
are exactly representable in bf16 and sums fit in fp32, so the result
is bit-exact vs the f32 reference.
"""

import sys

sys.path.insert(0, "/opt/trn_rl_repo")

import numpy as np

import concourse.bass as bass
import concourse.mybir as mybir
from concourse import bacc
from concourse.bass_utils import run_bass_kernel_spmd
from concourse.masks import make_identity
from concourse.tile import TileContext

P = 128
N_CORES = 8
N_IMG = 32
IMG_PER_CORE = N_IMG // N_CORES
C = 256
H = W = 56
HP = 58  # padded rows
WS = 64  # row stride of padded buffer (cols 0..57 used, 58..63 junk)
CHUNK = 8  # output rows per matmul tile -> N = 8*56 = 448

last_result = None  # stash of BassKernelResults for test harnesses


def build_conv_kernel():
    nc = bacc.Bacc()
    x_in = nc.declare_dram_parameter(
        "x", [IMG_PER_CORE, C, H, W], mybir.dt.float32, isOutput=False
    )
    w_in = nc.declare_dram_parameter("w", [C, C, 3, 3], mybir.dt.float32, isOutput=False)
    a_in = nc.declare_dram_parameter("alpha", [C, 1, 1], mybir.dt.float32, isOutput=False)
    y_out = nc.declare_dram_parameter(
        "y", [IMG_PER_CORE, C, H, W], mybir.dt.float32, isOutput=True
    )
    x_ap, w_ap, a_ap, y_ap = x_in[:], w_in[:], a_in[:], y_out[:]

    with TileContext(nc) as tc:
        with (
            tc.tile_pool(name="wpool", bufs=1) as wpool,
            tc.tile_pool(name="xpool", bufs=2) as xpool,
            tc.tile_pool(name="opool", bufs=4) as opool,
            tc.tile_pool(name="pp", bufs=4, space="PSUM") as pp,
        ):
            # ---- one-time weight prep ----
            ident = wpool.tile([P, P], mybir.dt.bfloat16, name="ident")
            make_identity(nc, ident)
            alpha_sb = wpool.tile([P, 2], mybir.dt.float32, name="alpha_sb")
            nc.sync.dma_start(
                out=alpha_sb, in_=a_ap.flatten().rearrange("(mt co) -> co mt", co=P)
            )
            # [ci_lo, cg, mt, pos, co]
            w_lhsT = wpool.tile([P, 2, 2, 9, P], mybir.dt.bfloat16, name="w_lhsT")
            for mt in range(2):
                wsrc = wpool.tile([P, C, 9], mybir.dt.float32, name="wsrc", bufs=2)
                nc.sync.dma_start(
                    out=wsrc,
                    in_=w_ap[mt * P : (mt + 1) * P].rearrange("co ci kh kw -> co ci (kh kw)"),
                )
                wsgn = wpool.tile([P, C, 9], mybir.dt.bfloat16, name="wsgn", bufs=2)
                nc.scalar.sign(wsgn, wsrc)
                for cg in range(2):
                    for pos in range(9):
                        tp = pp.tile([P, P], mybir.dt.bfloat16, name="tp", bufs=2)
                        nc.tensor.transpose(tp, wsgn[:, cg * P : (cg + 1) * P, pos], ident)
                        nc.vector.tensor_copy(out=w_lhsT[:, cg, mt, pos, :], in_=tp)

            # ---- main loop over images ----
            for img in range(IMG_PER_CORE):
                xpad = xpool.tile([P, 2, HP, WS], mybir.dt.bfloat16, name="xpad")
                nc.vector.memset(xpad[:, :, 0, 0:58], 0.0)
                nc.vector.memset(xpad[:, :, HP - 1, 0:58], 0.0)
                nc.vector.memset(xpad[:, :, 1 : HP - 1, 0], 0.0)
                nc.vector.memset(xpad[:, :, 1 : HP - 1, 57], 0.0)
                for cg in range(2):
                    xsrc = xpool.tile([P, H, W], mybir.dt.float32, name="xsrc")
                    nc.sync.dma_start(out=xsrc, in_=x_ap[img, cg * P : (cg + 1) * P])
                    nc.scalar.sign(xpad[:, cg, 1 : H + 1, 1 : W + 1], xsrc)

                for h0 in range(0, H, CHUNK):
                    for mt in range(2):
                        acc = pp.tile([P, CHUNK * W], mybir.dt.float32, name="acc")
                        k = 0
                        for cg in range(2):
                            for kh in range(3):
                                for kw in range(3):
                                    nc.tensor.matmul(
                                        acc,
                                        w_lhsT[:, cg, mt, kh * 3 + kw, :],
                                        xpad[:, cg, h0 + kh : h0 + kh + CHUNK, kw : kw + W],
                                        start=(k == 0),
                                        stop=(k == 17),
                                    )
                                    k += 1
                        ot = opool.tile([P, CHUNK * W], mybir.dt.float32, name="ot")
                        nc.vector.tensor_scalar_mul(
                            out=ot, in0=acc, scalar1=alpha_sb[:, mt : mt + 1]
                        )
                        nc.sync.dma_start(
                            out=y_ap[img, mt * P : (mt + 1) * P, h0 : h0 + CHUNK, :],
                            in_=ot.rearrange("co (h w) -> co h w", h=CHUNK),
                        )
    nc.compile()
    return nc


def kernel(x, weight, alpha, trace=False):
    global last_result
    x = np.ascontiguousarray(np.asarray(x, dtype=np.float32))
    weight = np.ascontiguousarray(np.asarray(weight, dtype=np.float32))
    alpha = np.ascontiguousarray(np.asarray(alpha, dtype=np.float32))

    nc = build_conv_kernel()
    in_maps = [
        {
            "x": np.ascontiguousarray(x[i * IMG_PER_CORE : (i + 1) * IMG_PER_CORE]),
            "w": weight,
            "alpha": alpha,
        }
        for i in range(N_CORES)
    ]
    res = run_bass_kernel_spmd(nc, in_maps, list(range(N_CORES)), trace=trace)
    last_result = res
    out = np.concatenate([res.results[i]["y"] for i in range(N_CORES)], axis=0)
    return out.astype(np.float32, copy=False)


# revision 5
# speedup vs baseline: 1.5279x; 1.5279x over previous
"""XNOR-Net conv2d kernel for Trainium2.

Computes conv2d(sign(x), sign(W), stride=1, pad=1) * alpha for
x:(32,256,56,56) f32, W:(256,256,3,3) f32, alpha:(256,1,1) f32.

Strategy: data-parallel over batch (4 images per core x 8 cores).
Per core, implicit GEMM on the PE array in fp8 (sign values +-1 are
exact in fp8e4; accumulation is fp32 in PSUM, sums are small ints, so
the result is bit-exact vs the f32 reference).

sign(x) lives in SBUF as a zero-padded fp8 image
[128 part = C_in%128, 2 c-groups, 59 rows, 64 row-stride]. Each 3x3
tap is one DoubleRow matmul contracting all 256 input channels
(K=128 partitions x 2 c-groups): lhsT [128, 2, 128co], rhs
[128, 2, 512] where the 512 is a flat window over 8 padded rows of
stride 64 (8 junk columns per row are computed but never copied out).
9 taps accumulate into one PSUM bank; copyback applies alpha.
"""

import sys

sys.path.insert(0, "/opt/trn_rl_repo")

import numpy as np

import concourse.bass as bass
import concourse.mybir as mybir
from concourse import bacc
from concourse.bass_utils import run_bass_kernel_spmd
from concourse.masks import make_identity
from concourse.tile import TileContext

P = 128
N_CORES = 8
N_IMG = 32
IMG_PER_CORE = N_IMG // N_CORES
C = 256
H = W = 56
HP = 58  # padded rows actually used (0..57)
HPA = 59  # allocated rows (guard row so flat windows stay in-bounds)
WS = 64  # row stride of padded buffer (cols 0..57 used)
CHUNK = 8  # output rows per matmul tile -> flat window N = 8*64 = 512
FP8 = mybir.dt.float8e4

last_result = None  # stash of BassKernelResults for test harnesses


def build_conv_kernel():
    nc = bacc.Bacc()
    x_in = nc.declare_dram_parameter(
        "x", [IMG_PER_CORE, C, H, W], mybir.dt.float32, isOutput=False
    )
    w_in = nc.declare_dram_parameter("w", [C, C, 3, 3], mybir.dt.float32, isOutput=False)
    a_in = nc.declare_dram_parameter("alpha", [C, 1, 1], mybir.dt.float32, isOutput=False)
    y_out = nc.declare_dram_parameter(
        "y", [IMG_PER_CORE, C, H, W], mybir.dt.float32, isOutput=True
    )
    x_ap, w_ap, a_ap, y_ap = x_in[:], w_in[:], a_in[:], y_out[:]

    with TileContext(nc) as tc:
        with (
            tc.tile_pool(name="wpool", bufs=1) as wpool,
            tc.tile_pool(name="xpool", bufs=2) as xpool,
            tc.tile_pool(name="opool", bufs=4) as opool,
            tc.tile_pool(name="pp", bufs=4, space="PSUM") as pp,
        ):
            # ---- one-time weight prep ----
            ident = wpool.tile([P, P], mybir.dt.bfloat16, name="ident")
            make_identity(nc, ident)
            alpha_sb = wpool.tile([P, 2], mybir.dt.float32, name="alpha_sb")
            nc.sync.dma_start(
                out=alpha_sb, in_=a_ap.flatten().rearrange("(mt co) -> co mt", co=P)
            )
            # [ci_lo, cg, mt, pos, co]
            w_bf16 = wpool.tile([P, 2, 2, 9, P], mybir.dt.bfloat16, name="w_bf16")
            w_lhsT = wpool.tile([P, 2, 2, 9, P], FP8, name="w_lhsT")
            for mt in range(2):
                wsrc = wpool.tile([P, C, 9], mybir.dt.float32, name="wsrc", bufs=2)
                nc.sync.dma_start(
                    out=wsrc,
                    in_=w_ap[mt * P : (mt + 1) * P].rearrange("co ci kh kw -> co ci (kh kw)"),
                )
                wsgn = wpool.tile([P, C, 9], mybir.dt.bfloat16, name="wsgn", bufs=2)
                nc.scalar.sign(wsgn, wsrc)
                for cg in range(2):
                    for pos in range(9):
                        tp = pp.tile([P, P], mybir.dt.bfloat16, name="tp", bufs=2)
                        nc.tensor.transpose(tp, wsgn[:, cg * P : (cg + 1) * P, pos], ident)
                        nc.vector.tensor_copy(out=w_bf16[:, cg, mt, pos, :], in_=tp)
            # single bf16 -> fp8 conversion of the whole weight block
            nc.vector.tensor_copy(out=w_lhsT, in_=w_bf16)

            # ---- main loop over images ----
            for img in range(IMG_PER_CORE):
                # flat layout [ci_lo, cg, HPA*WS]; rearranged views for 2D ops
                xpad = xpool.tile([P, 2, HPA * WS], FP8, name="xpad")
                xv = xpad.rearrange("p cg (r c) -> p cg r c", c=WS)
                nc.vector.memset(xv[:, :, 0, 0:58], 0.0)
                nc.vector.memset(xv[:, :, HP - 1, 0:58], 0.0)
                nc.vector.memset(xv[:, :, HP : HP + 1, 0:2], 0.0)  # guard-row wrap reads
                nc.vector.memset(xv[:, :, 1 : HP - 1, 0], 0.0)
                nc.vector.memset(xv[:, :, 1 : HP - 1, 57], 0.0)
                for cg in range(2):
                    xsrc = xpool.tile([P, H, W], mybir.dt.float32, name="xsrc")
                    nc.sync.dma_start(out=xsrc, in_=x_ap[img, cg * P : (cg + 1) * P])
                    nc.scalar.sign(xv[:, cg, 1 : H + 1, 1 : W + 1], xsrc)

                for h0 in range(0, H, CHUNK):
                    for mt in range(2):
                        acc = pp.tile([P, CHUNK * WS], mybir.dt.float32, name="acc")
                        k = 0
                        for kh in range(3):
                            for kw in range(3):
                                off = (h0 + kh) * WS + kw
                                nc.tensor.matmul(
                                    acc,
                                    w_lhsT[:, :, mt, kh * 3 + kw, :],
                                    xpad[:, :, off : off + CHUNK * WS],
                                    start=(k == 0),
                                    stop=(k == 8),
                                    perf_mode=mybir.MatmulPerfMode.DoubleRow,
                                )
                                k += 1
                        ot = opool.tile([P, CHUNK, W], mybir.dt.float32, name="ot")
                        nc.vector.tensor_scalar_mul(
                            out=ot,
                            in0=acc.rearrange("p (r c) -> p r c", c=WS)[:, :, :W],
                            scalar1=alpha_sb[:, mt : mt + 1],
                        )
                        nc.sync.dma_start(
                            out=y_ap[img, mt * P : (mt + 1) * P, h0 : h0 + CHUNK, :],
                            in_=ot,
                        )
    nc.compile()
    return nc


def kernel(x, weight, alpha, trace=False):
    global last_result
    x = np.ascontiguousarray(np.asarray(x, dtype=np.float32))
    weight = np.ascontiguousarray(np.asarray(weight, dtype=np.float32))
    alpha = np.ascontiguousarray(np.asarray(alpha, dtype=np.float32))

    nc = build_conv_kernel()
    in_maps = [
        {
            "x": np.ascontiguousarray(x[i * IMG_PER_CORE : (i + 1) * IMG_PER_CORE]),
            "w": weight,
            "alpha": alpha,
        }
        for i in range(N_CORES)
    ]
    res = run_bass_kernel_spmd(nc, in_maps, list(range(N_CORES)), trace=trace)
    last_result = res
    out = np.concatenate([res.results[i]["y"] for i in range(N_CORES)], axis=0)
    return out.astype(np.float32, copy=False)
